# revision 5
# baseline (speedup 1.0000x reference)
"""VQ codebook kernel for Trainium2 (8 NeuronCores, Bass/Tile).

Problem: features [131072, 128] f32, codes [2048, 128] f32.
Output: codes[argmin_k ||f - c_k||^2] -> [131072, 128] f32.

Strategy (data-parallel): shard features N across the 8 cores (16384 rows
each), replicate the codebook. argmin_k dist = argmax_k score, where
score = f.c_k - ||c_k||^2/2.

Host-side prep: features/codes are split hi/lo (fp32r = RNE to 11
mantissa bits; residuals additionally to fp8e5m2), features pre-transposed
per core. Per 128-row tile, entirely on chip:
  - PE (3 matmuls per 512-wide k-chunk, all accumulating into one
    [128,2048] PSUM tile):
      1. f_hi . c_hi          fp32r (1 cyc/row)
      2. f_lo . c_hi + f_hi . c_lo   one fp8e5 DoubleRow matmul
         (0.5 cyc/row; both operands' error products stay ~2^-16)
      3. bias: [1;1] x [b_hi; b_lo]  rank-2 fp32r matmul adds
         -csq/2 at ~2^-22 accuracy in one 1 cyc/row pass
    Total score error ~1e-4 abs -> ~a few argmax flips in 131072 rows.
  - DVE: one running-max tensor_tensor_scan over the PSUM tile -> r,
    row max M = r[:, -1].
  - ACT: idx = sum_t sign(M - r[t]) via one Sign activation with
    accum_out (count of positions before the first max = argmax index,
    matching jnp.argmin first-index tie-break).
  - GPSIMD indirect DMA gathers codes[idx] rows; DMA stores the tile.

Engine busy/tile ~ DVE 2.27us (cap), PE 2.13us, ACT 2.08us, Pool 1.2us.
"""

import os
import sys

import numpy as np

for _p in ("/opt/trn_rl_repo", "/root/.axon_site/_ro/trn_rl_repo"):
    if os.path.isdir(_p) and _p not in sys.path:
        sys.path.insert(0, _p)

import ml_dtypes

import concourse.bacc as bacc
import concourse.bass as bass
import concourse.mybir as mybir
import concourse.tile as tile
from concourse.bass_utils import run_bass_kernel_spmd

N, K, D = 131072, 2048, 128
N_CORES = 8
N_SHARD = N // N_CORES          # 16384
M_TILES = N_SHARD // 128        # 128
KC = 512                        # matmul chunk (one PSUM bank pair)
NEG_INF = -3.0e38
E5 = ml_dtypes.float8_e5m2

_compiled = None


def _rne11(x: np.ndarray) -> np.ndarray:
    """Round fp32 to fp32r: RNE to 11 mantissa bits (drop low 12)."""
    b = np.ascontiguousarray(x, dtype=np.float32).view(np.uint32)
    keep = np.uint32(0xFFFFF000)
    half = np.uint32(0x800)
    tie = (b >> np.uint32(12)) & np.uint32(1)
    r = (b + half - np.uint32(1) + tie) & keep
    return r.view(np.float32)


def _build(n_shard=N_SHARD, num_devices=N_CORES):
    m_tiles = n_shard // 128
    nc = bacc.Bacc("TRN2", target_bir_lowering=False, debug=False,
                   num_devices=num_devices)
    f32 = mybir.dt.float32
    f32r = mybir.dt.float32r
    f8 = mybir.dt.float8e5
    u32 = mybir.dt.uint32

    fhiT = nc.dram_tensor("fhiT", [D, n_shard], f32r,
                          kind="ExternalInput").ap()
    fw8 = nc.dram_tensor("fw8", [D, 2 * n_shard], f8,
                         kind="ExternalInput").ap()
    chiT = nc.dram_tensor("chiT", [D, K], f32r, kind="ExternalInput").ap()
    c8 = nc.dram_tensor("c8", [D, 2 * K], f8, kind="ExternalInput").ap()
    bstack = nc.dram_tensor("bstack", [2, K], f32r,
                            kind="ExternalInput").ap()
    ones2 = nc.dram_tensor("ones2", [2, 128], f32r,
                           kind="ExternalInput").ap()
    codes = nc.dram_tensor("codes", [K, D], f32, kind="ExternalInput").ap()
    out = nc.dram_tensor("out", [n_shard, D], f32, kind="ExternalOutput").ap()

    with tile.TileContext(nc) as tc:
        with (
            tc.tile_pool(name="const", bufs=1) as cp,
            tc.tile_pool(name="fin", bufs=3) as fin_pool,
            tc.tile_pool(name="rr", bufs=3) as r_pool,
            tc.tile_pool(name="jk", bufs=1) as junk_pool,
            tc.tile_pool(name="small", bufs=3) as small_pool,
            tc.tile_pool(name="gath", bufs=3) as gath_pool,
            tc.tile_pool(name="pdot", bufs=2, space="PSUM") as pdot_pool,
        ):
            chi_sb = cp.tile([D, K], f32r)
            c8_sb = cp.tile([D, 2 * K], f8)
            b_sb = cp.tile([2, K], f32r)
            ones2_sb = cp.tile([2, 128], f32r)
            ninf_sb = cp.tile([128, K], f32)
            for dst, src in [(chi_sb, chiT), (c8_sb, c8), (b_sb, bstack),
                             (ones2_sb, ones2)]:
                nc.sync.dma_start(dst[:], src[:])
            nc.gpsimd.memset(ninf_sb[:], NEG_INF)

            junk = junk_pool.tile([128, K], f32)

            for i in range(m_tiles):
                rows = slice(i * 128, (i + 1) * 128)
                fhi_t = fin_pool.tile([D, 128], f32r, tag="fhi")
                fw8_t = fin_pool.tile([D, 256], f8, tag="fw8")
                nc.sync.dma_start(fhi_t[:],
                                  fhiT[:, i * 128:(i + 1) * 128])
                nc.sync.dma_start(fw8_t[:],
                                  fw8[:, i * 256:(i + 1) * 256])
                fw8_3d = fw8_t[:].rearrange("p (two f) -> p two f", two=2)

                pdot = pdot_pool.tile([128, K], f32, tag="dot")
                for c in range(K // KC):
                    sl = slice(c * KC, (c + 1) * KC)
                    nc.tensor.matmul(pdot[:, sl], fhi_t[:], chi_sb[:, sl],
                                     start=True, stop=False)
                    x8 = c8_sb[:, 2 * c * KC:2 * (c + 1) * KC].rearrange(
                        "p (two f) -> p two f", two=2)
                    nc.tensor.matmul(pdot[:, sl], fw8_3d, x8,
                                     start=False, stop=False,
                                     perf_mode=mybir.MatmulPerfMode.DoubleRow)
                    nc.tensor.matmul(pdot[:, sl], ones2_sb[:], b_sb[:, sl],
                                     start=False, stop=True)

                # running max scan over the whole PSUM score tile
                r = r_pool.tile([128, K], f32, tag="r")
                nc.vector.tensor_tensor_scan(
                    out=r[:], data0=pdot[:],
                    data1=ninf_sb[:], initial=NEG_INF,
                    op0=mybir.AluOpType.max, op1=mybir.AluOpType.max)

                # ACT: idx = sum_t sign(M - r[t]),  M = r[:, -1]
                idxf = small_pool.tile([128, 1], f32, tag="idxf")
                nc.scalar.activation(
                    out=junk[:], in_=r[:],
                    func=mybir.ActivationFunctionType.Sign,
                    bias=r[:, K - 1:K], scale=-1.0, accum_out=idxf[:])

                idx_u = small_pool.tile([128, 1], u32, tag="idxu")
                nc.vector.tensor_copy(idx_u[:], idxf[:])

                gath = gath_pool.tile([128, D], f32)
                nc.gpsimd.indirect_dma_start(
                    out=gath[:],
                    out_offset=None,
                    in_=codes[:],
                    in_offset=bass.IndirectOffsetOnAxis(ap=idx_u[:, 0:1],
                                                        axis=0),
                )
                nc.sync.dma_start(out[rows, :], gath[:])
    nc.compile()
    return nc


def _get_compiled():
    global _compiled
    if _compiled is None:
        _compiled = _build()
    return _compiled


def kernel(features: np.ndarray, codes: np.ndarray,
           _trace: bool = False, _results_box: list | None = None
           ) -> np.ndarray:
    features = np.ascontiguousarray(features, dtype=np.float32)
    codes = np.ascontiguousarray(codes, dtype=np.float32)
    assert features.shape == (N, D) and codes.shape == (K, D)

    nc = _get_compiled()

    f_hi = _rne11(features)
    f_lo8 = (features - f_hi).astype(E5)
    f_hi8 = f_hi.astype(E5)
    c_hi = _rne11(codes)
    c_hi8 = c_hi.astype(E5)
    c_lo8 = (codes - c_hi).astype(E5)
    csq = (codes.astype(np.float64) ** 2).sum(axis=1)
    nh = (-0.5 * csq).astype(np.float32)
    b_hi = _rne11(nh)
    b_lo = _rne11(nh - b_hi)

    chiT = np.ascontiguousarray(c_hi.T)
    # c8: per 512-chunk, [c_hi8 | c_lo8] plane pairs: [128, 4, 2, 512]
    c8 = np.empty((D, 2 * K), dtype=E5)
    c8v = c8.reshape(D, K // KC, 2, KC)
    c8v[:, :, 0, :] = c_hi8.T.reshape(D, K // KC, KC)
    c8v[:, :, 1, :] = c_lo8.T.reshape(D, K // KC, KC)
    bstack = np.stack([b_hi, b_lo], axis=0)
    ones2 = np.ones((2, 128), dtype=np.float32)

    in_maps = []
    for c in range(N_CORES):
        sh = slice(c * N_SHARD, (c + 1) * N_SHARD)
        fhiT = np.ascontiguousarray(f_hi[sh].T)
        # fw8: per 128-row tile, [f_lo8.T | f_hi8.T] plane pairs
        fw8 = np.empty((D, 2 * N_SHARD), dtype=E5)
        fv = fw8.reshape(D, M_TILES, 2, 128)
        fv[:, :, 0, :] = f_lo8[sh].T.reshape(D, M_TILES, 128)
        fv[:, :, 1, :] = f_hi8[sh].T.reshape(D, M_TILES, 128)
        in_maps.append({
            "fhiT": fhiT,
            "fw8": fw8,
            "chiT": chiT,
            "c8": c8,
            "bstack": bstack,
            "ones2": ones2,
            "codes": codes,
        })
    res = run_bass_kernel_spmd(nc, in_maps, list(range(N_CORES)),
                               trace=_trace)
    if _results_box is not None:
        _results_box.append(res)
    out = np.concatenate([res.results[c]["out"] for c in range(N_CORES)],
                         axis=0)
    return out


if __name__ == "__main__":
    rng = np.random.default_rng(0)
    f = rng.standard_normal((N, D)).astype(np.float32)
    c = rng.standard_normal((K, D)).astype(np.float32)
    got = kernel(f, c)
    d = (f ** 2).sum(1)[:, None] - 2.0 * (f @ c.T) + (c ** 2).sum(1)
    want = c[np.argmin(d, axis=1)]
    err = np.abs(got - want)
    rel = np.linalg.norm(got - want) / np.linalg.norm(want)
    print(f"maxabs={err.max():.3e} rel={rel:.3e} "
          f"badrows={(err.max(1) > 1e-4).sum()}")


# revision 6
# speedup vs baseline: 1.0003x; 1.0003x over previous
"""VQ codebook kernel for Trainium2 (8 NeuronCores, Bass/Tile).

Problem: features [131072, 128] f32, codes [2048, 128] f32.
Output: codes[argmin_k ||f - c_k||^2] -> [131072, 128] f32.

Strategy (data-parallel): shard features N across the 8 cores (16384 rows
each), replicate the codebook. argmin_k dist = argmax_k score, where
score = f.c_k - ||c_k||^2/2.

Host-side prep: features/codes are split hi/lo (fp32r = RNE to 11
mantissa bits; residuals additionally to fp8e5m2), features pre-transposed
per core. Per 128-row tile, entirely on chip:
  - PE (3 matmuls per 512-wide k-chunk, all accumulating into one
    [128,2048] PSUM tile):
      1. f_hi . c_hi          fp32r (1 cyc/row)
      2. f_lo . c_hi + f_hi . c_lo   one fp8e5 DoubleRow matmul
         (0.5 cyc/row; both operands' error products stay ~2^-16)
      3. bias: [1;1] x [b_hi; b_lo]  rank-2 fp32r matmul adds
         -csq/2 at ~2^-22 accuracy in one 1 cyc/row pass
    Total score error ~1e-4 abs -> ~a few argmax flips in 131072 rows.
  - DVE: one running-max tensor_tensor_scan over the PSUM tile -> r,
    row max M = r[:, -1].
  - ACT: idx = sum_t sign(M - r[t]) via one Sign activation with
    accum_out (count of positions before the first max = argmax index,
    matching jnp.argmin first-index tie-break).
  - GPSIMD indirect DMA gathers codes[idx] rows; DMA stores the tile.

Engine busy/tile ~ DVE 2.27us (cap), PE 2.13us, ACT 2.08us, Pool 1.2us.
"""

import os
import sys

import numpy as np

for _p in ("/opt/trn_rl_repo", "/root/.axon_site/_ro/trn_rl_repo"):
    if os.path.isdir(_p) and _p not in sys.path:
        sys.path.insert(0, _p)

import ml_dtypes

import concourse.bacc as bacc
import concourse.bass as bass
import concourse.mybir as mybir
import concourse.tile as tile
from concourse.bass_utils import run_bass_kernel_spmd

N, K, D = 131072, 2048, 128
N_CORES = 8
N_SHARD = N // N_CORES          # 16384
M_TILES = N_SHARD // 128        # 128
KC = 512                        # matmul chunk (one PSUM bank pair)
NEG_INF = -3.0e38
E5 = ml_dtypes.float8_e5m2

_compiled = None


def _rne11(x: np.ndarray) -> np.ndarray:
    """Round fp32 to fp32r: RNE to 11 mantissa bits (drop low 12)."""
    b = np.ascontiguousarray(x, dtype=np.float32).view(np.uint32)
    keep = np.uint32(0xFFFFF000)
    half = np.uint32(0x800)
    tie = (b >> np.uint32(12)) & np.uint32(1)
    r = (b + half - np.uint32(1) + tie) & keep
    return r.view(np.float32)


def _build(n_shard=N_SHARD, num_devices=N_CORES):
    m_tiles = n_shard // 128
    nc = bacc.Bacc("TRN2", target_bir_lowering=False, debug=False,
                   num_devices=num_devices)
    f32 = mybir.dt.float32
    f32r = mybir.dt.float32r
    f8 = mybir.dt.float8e5
    u32 = mybir.dt.uint32

    fhiT = nc.dram_tensor("fhiT", [D, n_shard], f32r,
                          kind="ExternalInput").ap()
    fw8 = nc.dram_tensor("fw8", [D, 2 * n_shard], f8,
                         kind="ExternalInput").ap()
    chiT = nc.dram_tensor("chiT", [D, K], f32r, kind="ExternalInput").ap()
    c8 = nc.dram_tensor("c8", [D, 2 * K], f8, kind="ExternalInput").ap()
    bstack = nc.dram_tensor("bstack", [2, K], f32r,
                            kind="ExternalInput").ap()
    ones2 = nc.dram_tensor("ones2", [2, 128], f32r,
                           kind="ExternalInput").ap()
    codes = nc.dram_tensor("codes", [K, D], f32, kind="ExternalInput").ap()
    out = nc.dram_tensor("out", [n_shard, D], f32, kind="ExternalOutput").ap()

    with tile.TileContext(nc) as tc:
        with (
            tc.tile_pool(name="const", bufs=1) as cp,
            tc.tile_pool(name="fin", bufs=3) as fin_pool,
            tc.tile_pool(name="rr", bufs=2) as r_pool,
            tc.tile_pool(name="jk", bufs=1) as junk_pool,
            tc.tile_pool(name="small", bufs=3) as small_pool,
            tc.tile_pool(name="gath", bufs=3) as gath_pool,
            tc.tile_pool(name="pdot", bufs=2, space="PSUM") as pdot_pool,
        ):
            chi_sb = cp.tile([D, K], f32r)
            c8_sb = cp.tile([D, 2 * K], f8)
            b_sb = cp.tile([2, K], f32r)
            ones2_sb = cp.tile([2, 128], f32r)
            ninf_sb = cp.tile([128, K], f32)
            for dst, src in [(chi_sb, chiT), (c8_sb, c8), (b_sb, bstack),
                             (ones2_sb, ones2)]:
                nc.sync.dma_start(dst[:], src[:])
            nc.gpsimd.memset(ninf_sb[:], NEG_INF)

            junk = junk_pool.tile([128, K], f32)

            for i in range(m_tiles):
                rows = slice(i * 128, (i + 1) * 128)
                fhi_t = fin_pool.tile([D, 128], f32r, tag="fhi")
                fw8_t = fin_pool.tile([D, 256], f8, tag="fw8")
                nc.sync.dma_start(fhi_t[:],
                                  fhiT[:, i * 128:(i + 1) * 128])
                nc.sync.dma_start(fw8_t[:],
                                  fw8[:, i * 256:(i + 1) * 256])
                fw8_3d = fw8_t[:].rearrange("p (two f) -> p two f", two=2)

                pdot = pdot_pool.tile([128, K], f32, tag="dot")
                for c in range(K // KC):
                    sl = slice(c * KC, (c + 1) * KC)
                    nc.tensor.matmul(pdot[:, sl], fhi_t[:], chi_sb[:, sl],
                                     start=True, stop=False)
                    x8 = c8_sb[:, 2 * c * KC:2 * (c + 1) * KC].rearrange(
                        "p (two f) -> p two f", two=2)
                    nc.tensor.matmul(pdot[:, sl], fw8_3d, x8,
                                     start=False, stop=False,
                                     perf_mode=mybir.MatmulPerfMode.DoubleRow)
                    nc.tensor.matmul(pdot[:, sl], ones2_sb[:], b_sb[:, sl],
                                     start=False, stop=True)

                # running max scan over the whole PSUM score tile
                r = r_pool.tile([128, K], f32, tag="r")
                nc.vector.tensor_tensor_scan(
                    out=r[:], data0=pdot[:],
                    data1=ninf_sb[:], initial=NEG_INF,
                    op0=mybir.AluOpType.max, op1=mybir.AluOpType.max)

                # ACT: idx = sum_t sign(M - r[t]),  M = r[:, -1]
                idxf = small_pool.tile([128, 1], f32, tag="idxf")
                nc.scalar.activation(
                    out=junk[:], in_=r[:],
                    func=mybir.ActivationFunctionType.Sign,
                    bias=r[:, K - 1:K], scale=-1.0, accum_out=idxf[:])

                idx_u = small_pool.tile([128, 1], u32, tag="idxu")
                nc.vector.tensor_copy(idx_u[:], idxf[:])

                gath = gath_pool.tile([128, D], f32)
                nc.gpsimd.indirect_dma_start(
                    out=gath[:],
                    out_offset=None,
                    in_=codes[:],
                    in_offset=bass.IndirectOffsetOnAxis(ap=idx_u[:, 0:1],
                                                        axis=0),
                )
                nc.sync.dma_start(out[rows, :], gath[:])
    nc.compile()
    return nc


def _get_compiled():
    global _compiled
    if _compiled is None:
        _compiled = _build()
    return _compiled


def kernel(features: np.ndarray, codes: np.ndarray,
           _trace: bool = False, _results_box: list | None = None
           ) -> np.ndarray:
    features = np.ascontiguousarray(features, dtype=np.float32)
    codes = np.ascontiguousarray(codes, dtype=np.float32)
    assert features.shape == (N, D) and codes.shape == (K, D)

    nc = _get_compiled()

    f_hi = _rne11(features)
    f_lo8 = (features - f_hi).astype(E5)
    f_hi8 = f_hi.astype(E5)
    c_hi = _rne11(codes)
    c_hi8 = c_hi.astype(E5)
    c_lo8 = (codes - c_hi).astype(E5)
    csq = (codes.astype(np.float64) ** 2).sum(axis=1)
    nh = (-0.5 * csq).astype(np.float32)
    b_hi = _rne11(nh)
    b_lo = _rne11(nh - b_hi)

    chiT = np.ascontiguousarray(c_hi.T)
    # c8: per 512-chunk, [c_hi8 | c_lo8] plane pairs: [128, 4, 2, 512]
    c8 = np.empty((D, 2 * K), dtype=E5)
    c8v = c8.reshape(D, K // KC, 2, KC)
    c8v[:, :, 0, :] = c_hi8.T.reshape(D, K // KC, KC)
    c8v[:, :, 1, :] = c_lo8.T.reshape(D, K // KC, KC)
    bstack = np.stack([b_hi, b_lo], axis=0)
    ones2 = np.ones((2, 128), dtype=np.float32)

    in_maps = []
    for c in range(N_CORES):
        sh = slice(c * N_SHARD, (c + 1) * N_SHARD)
        fhiT = np.ascontiguousarray(f_hi[sh].T)
        # fw8: per 128-row tile, [f_lo8.T | f_hi8.T] plane pairs
        fw8 = np.empty((D, 2 * N_SHARD), dtype=E5)
        fv = fw8.reshape(D, M_TILES, 2, 128)
        fv[:, :, 0, :] = f_lo8[sh].T.reshape(D, M_TILES, 128)
        fv[:, :, 1, :] = f_hi8[sh].T.reshape(D, M_TILES, 128)
        in_maps.append({
            "fhiT": fhiT,
            "fw8": fw8,
            "chiT": chiT,
            "c8": c8,
            "bstack": bstack,
            "ones2": ones2,
            "codes": codes,
        })
    res = run_bass_kernel_spmd(nc, in_maps, list(range(N_CORES)),
                               trace=_trace)
    if _results_box is not None:
        _results_box.append(res)
    out = np.concatenate([res.results[c]["out"] for c in range(N_CORES)],
                         axis=0)
    return out


if __name__ == "__main__":
    rng = np.random.default_rng(0)
    f = rng.standard_normal((N, D)).astype(np.float32)
    c = rng.standard_normal((K, D)).astype(np.float32)
    got = kernel(f, c)
    d = (f ** 2).sum(1)[:, None] - 2.0 * (f @ c.T) + (c ** 2).sum(1)
    want = c[np.argmin(d, axis=1)]
    err = np.abs(got - want)
    rel = np.linalg.norm(got - want) / np.linalg.norm(want)
    print(f"maxabs={err.max():.3e} rel={rel:.3e} "
          f"badrows={(err.max(1) > 1e-4).sum()}")


# revision 7
# speedup vs baseline: 1.0079x; 1.0075x over previous
"""VQ codebook kernel for Trainium2 (8 NeuronCores, Bass/Tile).

Problem: features [131072, 128] f32, codes [2048, 128] f32.
Output: codes[argmin_k ||f - c_k||^2] -> [131072, 128] f32.

Strategy (data-parallel): shard features N across the 8 cores (16384 rows
each), replicate the codebook. argmin_k dist = argmax_k score, where
score = f.c_k - ||c_k||^2/2.

Host-side prep: features/codes are split hi/lo (fp32r = RNE to 11
mantissa bits; residuals additionally to fp8e5m2), features pre-transposed
per core. Per 128-row tile, entirely on chip:
  - PE (3 matmuls per 512-wide k-chunk, all accumulating into one
    [128,2048] PSUM tile):
      1. f_hi . c_hi          fp32r (1 cyc/row)
      2. f_lo . c_hi + f_hi . c_lo   one fp8e5 DoubleRow matmul
         (0.5 cyc/row; both operands' error products stay ~2^-16)
      3. bias: [1;1] x [b_hi; b_lo]  rank-2 fp32r matmul adds
         -csq/2 at ~2^-22 accuracy in one 1 cyc/row pass
    Total score error ~1e-4 abs -> ~a few argmax flips in 131072 rows.
  - DVE: one running-max tensor_tensor_scan over the PSUM tile -> r,
    row max M = r[:, -1].
  - ACT: idx = sum_t sign(M - r[t]) via one Sign activation with
    accum_out (count of positions before the first max = argmax index,
    matching jnp.argmin first-index tie-break).
  - GPSIMD indirect DMA gathers codes[idx] rows; DMA stores the tile.

Engine busy/tile ~ DVE 2.27us (cap), PE 2.13us, ACT 2.08us, Pool 1.2us.
"""

import os
import sys

import numpy as np

for _p in ("/opt/trn_rl_repo", "/root/.axon_site/_ro/trn_rl_repo"):
    if os.path.isdir(_p) and _p not in sys.path:
        sys.path.insert(0, _p)

import ml_dtypes

import concourse.bacc as bacc
import concourse.bass as bass
import concourse.mybir as mybir
import concourse.tile as tile
from concourse.bass_utils import run_bass_kernel_spmd

N, K, D = 131072, 2048, 128
N_CORES = 8
N_SHARD = N // N_CORES          # 16384
M_TILES = N_SHARD // 128        # 128
KC = 512                        # matmul chunk (one PSUM bank pair)
NEG_INF = -3.0e38
E5 = ml_dtypes.float8_e5m2

_compiled = None


def _rne11(x: np.ndarray) -> np.ndarray:
    """Round fp32 to fp32r: RNE to 11 mantissa bits (drop low 12)."""
    b = np.ascontiguousarray(x, dtype=np.float32).view(np.uint32)
    keep = np.uint32(0xFFFFF000)
    half = np.uint32(0x800)
    tie = (b >> np.uint32(12)) & np.uint32(1)
    r = (b + half - np.uint32(1) + tie) & keep
    return r.view(np.float32)


def _build(n_shard=N_SHARD, num_devices=N_CORES):
    m_tiles = n_shard // 128
    nc = bacc.Bacc("TRN2", target_bir_lowering=False, debug=False,
                   num_devices=num_devices)
    f32 = mybir.dt.float32
    f32r = mybir.dt.float32r
    f8 = mybir.dt.float8e5
    u32 = mybir.dt.uint32

    fhiT = nc.dram_tensor("fhiT", [D, n_shard], f32r,
                          kind="ExternalInput").ap()
    fw8 = nc.dram_tensor("fw8", [D, 2 * n_shard], f8,
                         kind="ExternalInput").ap()
    chiT = nc.dram_tensor("chiT", [D, K], f32r, kind="ExternalInput").ap()
    c8 = nc.dram_tensor("c8", [D, 2 * K], f8, kind="ExternalInput").ap()
    bstack = nc.dram_tensor("bstack", [2, K], f32r,
                            kind="ExternalInput").ap()
    ones2 = nc.dram_tensor("ones2", [2, 128], f32r,
                           kind="ExternalInput").ap()
    codes = nc.dram_tensor("codes", [K, D], f32, kind="ExternalInput").ap()
    out = nc.dram_tensor("out", [n_shard, D], f32, kind="ExternalOutput").ap()

    with tile.TileContext(nc) as tc:
        with (
            tc.tile_pool(name="const", bufs=1) as cp,
            tc.tile_pool(name="fin", bufs=3) as fin_pool,
            tc.tile_pool(name="rr", bufs=2) as r_pool,
            tc.tile_pool(name="jk", bufs=1) as junk_pool,
            tc.tile_pool(name="small", bufs=3) as small_pool,
            tc.tile_pool(name="gath", bufs=3) as gath_pool,
            tc.tile_pool(name="pdot", bufs=2, space="PSUM") as pdot_pool,
        ):
            chi_sb = cp.tile([D, K], f32r)
            c8_sb = cp.tile([D, 2 * K], f8)
            b_sb = cp.tile([2, K], f32r)
            ones2_sb = cp.tile([2, 128], f32r)
            ninf_sb = cp.tile([128, K], f32)
            # fill optimization: stream the codebook constants in halves with
            # the first two tiles' feature loads slotted between, so PE can
            # start on chunks 0-1 while the second half is still in flight
            nc.sync.dma_start(chi_sb[:, 0:K // 2], chiT[:, 0:K // 2])
            nc.sync.dma_start(c8_sb[:, 0:K], c8[:, 0:K])
            nc.sync.dma_start(b_sb[:], bstack[:])
            nc.sync.dma_start(ones2_sb[:], ones2[:])
            pre = []
            for i in range(2):
                fhi_p = fin_pool.tile([D, 128], f32r, tag="fhi")
                fw8_p = fin_pool.tile([D, 256], f8, tag="fw8")
                nc.sync.dma_start(fhi_p[:], fhiT[:, i * 128:(i + 1) * 128])
                nc.sync.dma_start(fw8_p[:], fw8[:, i * 256:(i + 1) * 256])
                pre.append((fhi_p, fw8_p))
            nc.sync.dma_start(chi_sb[:, K // 2:K], chiT[:, K // 2:K])
            nc.sync.dma_start(c8_sb[:, K:2 * K], c8[:, K:2 * K])
            nc.gpsimd.memset(ninf_sb[:], NEG_INF)

            junk = junk_pool.tile([128, K], f32)

            for i in range(m_tiles):
                rows = slice(i * 128, (i + 1) * 128)
                if i < 2:
                    fhi_t, fw8_t = pre[i]
                else:
                    fhi_t = fin_pool.tile([D, 128], f32r, tag="fhi")
                    fw8_t = fin_pool.tile([D, 256], f8, tag="fw8")
                    nc.sync.dma_start(fhi_t[:],
                                      fhiT[:, i * 128:(i + 1) * 128])
                    nc.sync.dma_start(fw8_t[:],
                                      fw8[:, i * 256:(i + 1) * 256])
                fw8_3d = fw8_t[:].rearrange("p (two f) -> p two f", two=2)

                pdot = pdot_pool.tile([128, K], f32, tag="dot")
                for c in range(K // KC):
                    sl = slice(c * KC, (c + 1) * KC)
                    nc.tensor.matmul(pdot[:, sl], fhi_t[:], chi_sb[:, sl],
                                     start=True, stop=False)
                    x8 = c8_sb[:, 2 * c * KC:2 * (c + 1) * KC].rearrange(
                        "p (two f) -> p two f", two=2)
                    nc.tensor.matmul(pdot[:, sl], fw8_3d, x8,
                                     start=False, stop=False,
                                     perf_mode=mybir.MatmulPerfMode.DoubleRow)
                    nc.tensor.matmul(pdot[:, sl], ones2_sb[:], b_sb[:, sl],
                                     start=False, stop=True)

                # running max scan over the whole PSUM score tile
                r = r_pool.tile([128, K], f32, tag="r")
                nc.vector.tensor_tensor_scan(
                    out=r[:], data0=pdot[:],
                    data1=ninf_sb[:], initial=NEG_INF,
                    op0=mybir.AluOpType.max, op1=mybir.AluOpType.max)

                # ACT: idx = sum_t sign(M - r[t]),  M = r[:, -1]
                idxf = small_pool.tile([128, 1], f32, tag="idxf")
                nc.scalar.activation(
                    out=junk[:], in_=r[:],
                    func=mybir.ActivationFunctionType.Sign,
                    bias=r[:, K - 1:K], scale=-1.0, accum_out=idxf[:])

                idx_u = small_pool.tile([128, 1], u32, tag="idxu")
                nc.vector.tensor_copy(idx_u[:], idxf[:])

                gath = gath_pool.tile([128, D], f32)
                nc.gpsimd.indirect_dma_start(
                    out=gath[:],
                    out_offset=None,
                    in_=codes[:],
                    in_offset=bass.IndirectOffsetOnAxis(ap=idx_u[:, 0:1],
                                                        axis=0),
                )
                nc.sync.dma_start(out[rows, :], gath[:])
    nc.compile()
    return nc


def _get_compiled():
    global _compiled
    if _compiled is None:
        _compiled = _build()
    return _compiled


def kernel(features: np.ndarray, codes: np.ndarray,
           _trace: bool = False, _results_box: list | None = None
           ) -> np.ndarray:
    features = np.ascontiguousarray(features, dtype=np.float32)
    codes = np.ascontiguousarray(codes, dtype=np.float32)
    assert features.shape == (N, D) and codes.shape == (K, D)

    nc = _get_compiled()

    f_hi = _rne11(features)
    f_lo8 = (features - f_hi).astype(E5)
    f_hi8 = f_hi.astype(E5)
    c_hi = _rne11(codes)
    c_hi8 = c_hi.astype(E5)
    c_lo8 = (codes - c_hi).astype(E5)
    csq = (codes.astype(np.float64) ** 2).sum(axis=1)
    nh = (-0.5 * csq).astype(np.float32)
    b_hi = _rne11(nh)
    b_lo = _rne11(nh - b_hi)

    chiT = np.ascontiguousarray(c_hi.T)
    # c8: per 512-chunk, [c_hi8 | c_lo8] plane pairs: [128, 4, 2, 512]
    c8 = np.empty((D, 2 * K), dtype=E5)
    c8v = c8.reshape(D, K // KC, 2, KC)
    c8v[:, :, 0, :] = c_hi8.T.reshape(D, K // KC, KC)
    c8v[:, :, 1, :] = c_lo8.T.reshape(D, K // KC, KC)
    bstack = np.stack([b_hi, b_lo], axis=0)
    ones2 = np.ones((2, 128), dtype=np.float32)

    in_maps = []
    for c in range(N_CORES):
        sh = slice(c * N_SHARD, (c + 1) * N_SHARD)
        fhiT = np.ascontiguousarray(f_hi[sh].T)
        # fw8: per 128-row tile, [f_lo8.T | f_hi8.T] plane pairs
        fw8 = np.empty((D, 2 * N_SHARD), dtype=E5)
        fv = fw8.reshape(D, M_TILES, 2, 128)
        fv[:, :, 0, :] = f_lo8[sh].T.reshape(D, M_TILES, 128)
        fv[:, :, 1, :] = f_hi8[sh].T.reshape(D, M_TILES, 128)
        in_maps.append({
            "fhiT": fhiT,
            "fw8": fw8,
            "chiT": chiT,
            "c8": c8,
            "bstack": bstack,
            "ones2": ones2,
            "codes": codes,
        })
    res = run_bass_kernel_spmd(nc, in_maps, list(range(N_CORES)),
                               trace=_trace)
    if _results_box is not None:
        _results_box.append(res)
    out = np.concatenate([res.results[c]["out"] for c in range(N_CORES)],
                         axis=0)
    return out


if __name__ == "__main__":
    rng = np.random.default_rng(0)
    f = rng.standard_normal((N, D)).astype(np.float32)
    c = rng.standard_normal((K, D)).astype(np.float32)
    got = kernel(f, c)
    d = (f ** 2).sum(1)[:, None] - 2.0 * (f @ c.T) + (c ** 2).sum(1)
    want = c[np.argmin(d, axis=1)]
    err = np.abs(got - want)
    rel = np.linalg.norm(got - want) / np.linalg.norm(want)
    print(f"maxabs={err.max():.3e} rel={rel:.3e} "
          f"badrows={(err.max(1) > 1e-4).sum()}")
